# revision 10
# baseline (speedup 1.0000x reference)
"""Multi-head cross-attention Trainium2 Bass kernel, SPMD over 8 NeuronCores.

Sharding: core c handles batch b = c//2 and head group g = c%2 (8 of 16 heads).
Each core computes a partial output projection (its heads' W_o rows); the host
sums the two partials per batch element.

Device pipeline per core (all matmuls bf16 with fp32 PSUM accumulation):
  kT = (Wk^T x^T)          [512 hd, 2048 kseq]   (per-partition bias b_k)
  v  = (x Wv)              [2048 kseq, 8*65]     (65th col per head = ones)
  qT = (Wq^T y^T)          [512 hd, 1024 q]      (per-partition bias b_q)
  per (head-pair, q-tile, k-chunk):
      S^T[k, q|q'] = kT_h^T-chunk @ qT_h for both heads of the pair
        (K=64 row-tiled at partitions 0/64 -> the two matmuls run
         concurrently in the PE array; both write one 2-bank PSUM tile)
      em = exp(0.125 * S^T)  (one ACT op per pair; no row-max: |S|<=~25)
      em *= maskT            (DVE mostly, 1/4 on GPSIMD to balance engines)
      acc_h[65, q] += [v_h | 1]^T @ em_h   (row 64 = softmax denominator)
  normalize: vals_h = acc[0:64] * bcast(1/acc[64])  (PE outer-product bcast,
      reciprocal_approx_fast; deferred into the next head-pair's loop)
  out_partial = vals^T-chunks @ Wo-rows  -> [1024 q, 1024 D] fp32
b_v and b_o fold into a host-side constant row (attn rows sum to 1).

Scheduling: the PE stream (~168us effective) is the pacing resource; ACT
(exp, ~140us) and DVE (~135us) hide underneath IF the pipeline starts early
and stays dense.  Unlike the previous revision (which ran ALL of v/kT0/qT0
before the first score, idling ACT+DVE for ~50us), this version emits only
the minimal prefix (kT0-n0, qT0-n0, v0) and drains every other projection
chain and the first-half output chains through a deadline-driven filler
queue inside the attention loop, so cross-engine overlap begins ~8us in.
DMA issues in need-order with column-sliced weight loads to unblock the
first chains ASAP.  Output copies run on GPSIMD to keep ACT = exp only.
"""

import sys
from collections import deque

import numpy as np
import ml_dtypes

if "/opt/trn_rl_repo" not in sys.path:
    sys.path.insert(0, "/opt/trn_rl_repo")

BF = ml_dtypes.bfloat16

B, NKV, NQ, D, H = 4, 2048, 1024, 1024, 16
HD = D // H          # 64
NHL = 8              # heads per core (local)
P = 128
DC = D // P          # 8 contraction chunks over model dim
KC = NKV // P        # 16 key-seq chunks
QT = NQ // 512       # 2 q tiles of 512 for attention
MT = 4               # hd-dim chunks of kT/qT (512/128)
NSLOT = NHL // 2 * QT * KC   # 128 (hp, t, kc) attention slots

_CACHE = {}


def _build_program():
    import concourse.bass as bass
    import concourse.mybir as mybir
    import concourse.tile as tile
    from concourse import bacc

    f32 = mybir.dt.float32
    bf16 = mybir.dt.bfloat16

    nc = bacc.Bacc(
        "TRN2", target_bir_lowering=False, debug=False, num_devices=8
    )

    xT_d = nc.dram_tensor("xT", [D, NKV], bf16, kind="ExternalInput").ap()
    yT_d = nc.dram_tensor("yT", [D, NQ], bf16, kind="ExternalInput").ap()
    maskT_d = nc.dram_tensor("maskT", [NKV, NQ], bf16, kind="ExternalInput").ap()
    wk_d = nc.dram_tensor("wk", [D, 512], bf16, kind="ExternalInput").ap()
    wv_d = nc.dram_tensor("wv", [D, 512], bf16, kind="ExternalInput").ap()
    wq_d = nc.dram_tensor("wq", [D, 512], bf16, kind="ExternalInput").ap()
    wo_d = nc.dram_tensor("wo", [512, D], bf16, kind="ExternalInput").ap()
    bk_d = nc.dram_tensor("bk", [512, 1], f32, kind="ExternalInput").ap()
    bq_d = nc.dram_tensor("bq", [512, 1], f32, kind="ExternalInput").ap()
    out_d = nc.dram_tensor("out", [NQ, D], bf16, kind="ExternalOutput").ap()

    Exp = mybir.ActivationFunctionType.Exp

    with tile.TileContext(nc) as tc:
        with (
            tc.tile_pool(name="persist", bufs=1) as persist,
            tc.tile_pool(name="work", bufs=3) as work,
            tc.tile_pool(name="empool", bufs=5) as empool,
            tc.tile_pool(name="pmm", bufs=2, space="PSUM") as pmm,
            tc.tile_pool(name="pacc", bufs=2, space="PSUM") as pacc,
            tc.tile_pool(name="psc", bufs=2, space="PSUM") as psc,
        ):
            def row_tile(nchunks, cols, dtype, label):
                return [
                    persist.tile(
                        [P, cols], dtype, tag=f"{label}{i}", name=f"{label}{i}"
                    )
                    for i in range(nchunks)
                ]

            def load(tiles, dram, i):
                nc.sync.dma_start(tiles[i], dram[i * P:(i + 1) * P, :])

            wk_sb = row_tile(DC, 512, bf16, "wk")
            wv_sb = row_tile(DC, 512, bf16, "wv")
            xT_sb = row_tile(DC, NKV, bf16, "xT")
            wq_sb = row_tile(DC, 512, bf16, "wq")
            yT_sb = row_tile(DC, NQ, bf16, "yT")
            bk_sb = row_tile(MT, 1, f32, "bk")
            bq_sb = row_tile(MT, 1, f32, "bq")
            maskT_sb = row_tile(KC, NQ, bf16, "mT")
            wo_sb = row_tile(MT, D, bf16, "wo")

            def load_cols(tiles, dram, d, c0, c1):
                nc.sync.dma_start(
                    tiles[d][:, c0:c1], dram[d * P:(d + 1) * P, c0:c1]
                )

            # ---- DMA issue order == consumption order ----
            # Minimal prefix for the first scores / attn matmuls, then the
            # rest interleaved so each filler chain's last input arrives
            # just before its deadline slot.
            for d in range(DC):
                load_cols(wk_sb, wk_d, d, 0, P)       # wk m0 cols only
                load_cols(xT_sb, xT_d, d, 0, 512)     # xT q-quarter 0
            for d in range(DC):
                load_cols(wq_sb, wq_d, d, 0, P)       # wq m0 cols only
                load_cols(yT_sb, yT_d, d, 0, 512)     # yT first q-half
            for d in range(DC):
                load(wv_sb, wv_d, d)                  # wv full (v chains)
            for m in range(MT):
                load(bk_sb, bk_d, m)
                load(bq_sb, bq_d, m)
            for i in range(0, 2):
                load(maskT_sb, maskT_d, i)
            for d in range(DC):
                load_cols(xT_sb, xT_d, d, 512, 1024)  # xT quarter 1
            for i in range(2, 6):
                load(maskT_sb, maskT_d, i)
            for d in range(DC):
                load_cols(wk_sb, wk_d, d, P, 512)     # wk m1-3
            for i in range(6, 8):
                load(maskT_sb, maskT_d, i)
            for d in range(DC):
                load_cols(xT_sb, xT_d, d, 1024, 1536)  # xT quarter 2
            for i in range(8, 10):
                load(maskT_sb, maskT_d, i)
            for d in range(DC):
                load_cols(wq_sb, wq_d, d, P, 512)     # wq m1-3
                load_cols(yT_sb, yT_d, d, 512, 1024)  # yT second q-half
            for i in range(10, 12):
                load(maskT_sb, maskT_d, i)
            for d in range(DC):
                load_cols(xT_sb, xT_d, d, 1536, 2048)  # xT quarter 3
            for i in range(12, 16):
                load(maskT_sb, maskT_d, i)
            for m in range(MT):
                load(wo_sb, wo_d, m)

            ones_sb = persist.tile([1, HD], bf16, tag="ones", name="ones")
            nc.gpsimd.memset(ones_sb, 1.0)

            kT_sb = [
                persist.tile([P, NKV], bf16, tag=f"kT{m}", name=f"kT{m}")
                for m in range(MT)
            ]
            qT_sb = [
                persist.tile([P, NQ], bf16, tag=f"qT{m}", name=f"qT{m}")
                for m in range(MT)
            ]
            v_sb = [
                persist.tile([P, NHL * 65], bf16, tag=f"v{i}", name=f"v{i}")
                for i in range(KC)
            ]
            for i in range(KC):
                nc.gpsimd.memset(
                    v_sb[i].rearrange("p (h c) -> p h c", c=65)[:, :, 64:65], 1.0
                )
            vals_sb = [
                persist.tile([P, NQ], bf16, tag=f"vals{c}", name=f"vals{c}")
                for c in range(MT)
            ]

            # ---- chain builders (lists of single-matmul closures) ----
            def proj_chain(m, n, which):
                """kT/qT chunk-m column-chain n: 8 matmuls + bias-add."""
                w_sb, dst, bias = (
                    (wk_sb, kT_sb, bk_sb) if which == "k"
                    else (wq_sb, qT_sb, bq_sb)
                )
                hold = {}
                ops = []
                for d in range(DC):
                    def op(d=d):
                        if d == 0:
                            hold[0] = pmm.tile(
                                [P, 512], f32, tag="mm",
                                name=f"pj{which}{m}_{n}"
                            )
                        nc.tensor.matmul(
                            hold[0],
                            lhsT=w_sb[d][:, m * P:(m + 1) * P],
                            rhs=(xT_sb if which == "k" else yT_sb)[d][
                                :, n * 512:(n + 1) * 512],
                            start=(d == 0),
                            stop=(d == DC - 1),
                        )
                        if d == DC - 1:
                            nc.vector.tensor_scalar_add(
                                dst[m][:, n * 512:(n + 1) * 512],
                                hold[0], bias[m]
                            )
                    ops.append(op)
                return ops

            def v_chain(i):
                """v row-chunk i: 8 matmuls + PSUM->SBUF copy."""
                hold = {}
                ops = []
                for d in range(DC):
                    def op(d=d, i=i):
                        if d == 0:
                            hold[0] = pmm.tile(
                                [P, 512], f32, tag="mm", name=f"ps_v{i}"
                            )
                        nc.tensor.matmul(
                            hold[0],
                            lhsT=xT_sb[d][:, i * P:(i + 1) * P],
                            rhs=wv_sb[d],
                            start=(d == 0),
                            stop=(d == DC - 1),
                        )
                        if d == DC - 1:
                            v3 = v_sb[i].rearrange("p (h c) -> p h c", c=65)
                            nc.vector.tensor_copy(
                                v3[:, :, 0:64],
                                hold[0].rearrange("p (h c) -> p h c", c=64),
                            )
                    ops.append(op)
                return ops

            def wo_chain(t2, n):
                """output-projection chain: 4 matmuls + copy (GPSIMD) + DMA."""
                hold = {}
                ops = []
                for c in range(MT):
                    def op(c=c, t2=t2, n=n):
                        if c == 0:
                            hold[0] = pmm.tile(
                                [P, 512], f32, tag="mm", name=f"ps_o{t2}_{n}"
                            )
                        nc.tensor.matmul(
                            hold[0],
                            lhsT=vals_sb[c][:, t2 * P:(t2 + 1) * P],
                            rhs=wo_sb[c][:, n * 512:(n + 1) * 512],
                            start=(c == 0),
                            stop=(c == MT - 1),
                        )
                        if c == MT - 1:
                            ot = work.tile(
                                [P, 512], bf16, tag="ot",
                                name=f"ot{t2}_{n}", bufs=3
                            )
                            # GPSIMD cannot read PSUM; DVE has the headroom
                            nc.vector.tensor_copy(ot, hold[0])
                            nc.sync.dma_start(
                                out_d[t2 * P:(t2 + 1) * P,
                                      n * 512:(n + 1) * 512], ot
                            )
                    ops.append(op)
                return ops

            # ---- deadline-sorted filler queue ----
            # Each entry: (deadline_slot, op).  Popped (and emitted) when the
            # attention loop reaches that slot; order within equal deadlines
            # follows append order, which is dependency-safe.
            pend = []

            def add_chain(deadline, ops):
                for op in ops:
                    pend.append((deadline, op))

            # v[i] first consumed at slot i (block 0 covers kc 0..15); its
            # attn matmul trails the slot's scores by ~1.5 slots of
            # cross-engine latency, so deadline i still arrives in time
            for i in range(1, KC):
                add_chain(i, v_chain(i))
            # kT chunk 0 column-chains n>=1: consumed at slot 4n
            for n in range(1, 4):
                add_chain(4 * n - 2, proj_chain(0, n, "k"))
            # qT chunk 0 n=1: consumed from slot 16
            add_chain(13, proj_chain(0, 1, "q"))
            # chunks m>=1: block (m, t) starts at slot 32m + 16t
            for m in range(1, MT):
                for n in range(4):
                    add_chain(32 * m + 4 * n - 3, proj_chain(m, n, "k"))
                for t in range(QT):
                    add_chain(32 * m + 16 * t - 3, proj_chain(m, t, "q"))
            # first-half output chains: vals q<512 final once block (3,0)'s
            # norms have popped (slots 112 and 114 of block (3,1)); spread
            # the 8 chains through the rest of block (3,1)
            for j, (t2, n) in enumerate(
                [(t2, n) for t2 in range(4) for n in range(2)]
            ):
                add_chain(115 + (3 * j) // 2, wo_chain(t2, n))
            pend.sort(key=lambda e: e[0])
            pend = deque(pend)

            def drain(s):
                while pend and pend[0][0] <= s:
                    pend.popleft()[1]()

            # ---- minimal prefix: first scores/attn inputs ----
            for op in proj_chain(0, 0, "k"):
                op()
            for op in proj_chain(0, 0, "q"):
                op()
            for op in v_chain(0):
                op()

            # ---- attention (fillers + deferred normalize interleaved) ----
            norm_pending = deque()

            def make_norm(hp, t, a, h, ut, s_f):
                po = a * HD
                qs = slice(t * 512, (t + 1) * 512)

                def norm_op():
                    r_f = work.tile([1, 512], f32, tag="r", name=f"r{h}_{t}")
                    nc.vector.reciprocal_approx_fast(r_f, s_f)
                    r_b = work.tile([1, 512], bf16, tag="rb", name=f"rb{h}_{t}")
                    nc.vector.tensor_copy(r_b, r_f)
                    bps = pmm.tile([HD, 512], f32, tag="mm", name=f"bps{h}_{t}")
                    nc.tensor.matmul(
                        bps, lhsT=ones_sb, rhs=r_b, start=True, stop=True
                    )
                    nc.vector.tensor_mul(vals_sb[hp][po:po + HD, qs], ut, bps)
                return norm_op

            for hp in range(NHL // 2):
                h0, h1 = 2 * hp, 2 * hp + 1
                for t in range(QT):
                    qs = slice(t * 512, (t + 1) * 512)
                    accs = [
                        pacc.tile([65, 512], f32, tag="acc", name=f"acc{h}_{t}")
                        for h in (h0, h1)
                    ]
                    for kc in range(KC):
                        s = (hp * QT + t) * KC + kc
                        sp2 = psc.tile(
                            [P, 1024], f32, tag="sc", name=f"sp{hp}_{t}_{kc}"
                        )
                        for a in range(2):
                            po = a * HD
                            nc.tensor.matmul(
                                sp2[:, a * 512:(a + 1) * 512],
                                lhsT=kT_sb[hp][po:po + HD, kc * P:(kc + 1) * P],
                                rhs=qT_sb[hp][po:po + HD, qs],
                                start=True,
                                stop=True,
                            )
                        em2 = empool.tile(
                            [P, 1024], bf16, tag="em", name=f"em{hp}_{t}_{kc}"
                        )
                        nc.scalar.activation(em2, sp2, Exp, scale=0.125)
                        # one masked multiply for both heads: the mask chunk is
                        # read once and broadcast (step-0 dim) over the pair;
                        # alternate DVE/GPSIMD by chunk parity (both operands
                        # SBUF, the one space GPSIMD may touch) to split the
                        # heaviest element-wise load across two engines
                        mb = (maskT_sb[kc][:, qs]
                              .rearrange("p (o q) -> p o q", o=1)
                              .broadcast_to([P, 2, 512]))
                        em3 = em2.rearrange("p (o q) -> p o q", o=2)
                        if kc % 2 == 1:
                            nc.gpsimd.tensor_mul(em3, em3, mb)
                        else:
                            nc.vector.tensor_mul(em3, em3, mb)
                        # fillers go between the mask op and the attn matmuls:
                        # the PE works through them while DVE/GPSIMD finish
                        # the mask, and scores never queue behind a filler
                        # that may itself be waiting on a DMA
                        drain(s)
                        if kc in (0, 2) and norm_pending:
                            norm_pending.popleft()()
                        for a, h in enumerate((h0, h1)):
                            nc.tensor.matmul(
                                accs[a],
                                lhsT=v_sb[kc][:, h * 65:(h + 1) * 65],
                                rhs=em2[:, a * 512:(a + 1) * 512],
                                start=(kc == 0),
                                stop=(kc == KC - 1),
                            )
                    for a, h in enumerate((h0, h1)):
                        acc = accs[a]
                        # free the PSUM accumulator quickly (copies only);
                        # the rest of the normalization is deferred into the
                        # next block's loop.
                        ut = work.tile(
                            [HD, 512], f32, tag="ut", name=f"ut{h}_{t}", bufs=5
                        )
                        nc.vector.tensor_copy(ut, acc[0:HD, :])
                        s_f = work.tile(
                            [1, 512], f32, tag="s", name=f"s{h}_{t}", bufs=5
                        )
                        nc.scalar.copy(s_f, acc[64:65, :])
                        norm_pending.append(make_norm(hp, t, a, h, ut, s_f))

            # ---- tail: last block's norms, then second-half out chains ----
            while norm_pending:
                norm_pending.popleft()()
            drain(NSLOT)               # any stragglers (should be none)
            for t2 in range(4, NQ // P):
                for n in range(2):
                    for op in wo_chain(t2, n):
                        op()

    nc.compile()
    return nc


def _get_program():
    if "nc" not in _CACHE:
        _CACHE["nc"] = _build_program()
    return _CACHE["nc"]


def _per_core_inputs(x, y, mask, W_kv, b_kv, W_q, b_q, W_o):
    """Build the 8 per-core input maps."""
    in_maps = []
    mask_f = mask.astype(np.float32)
    for c in range(8):
        b, g = c // 2, c % 2
        gh = np.arange(g * 8, g * 8 + 8)
        k_cols = (gh[:, None] * 2 * HD + np.arange(HD)[None, :]).ravel()
        v_cols = k_cols + HD
        q_cols = slice(g * 512, (g + 1) * 512)
        in_maps.append({
            "xT": np.ascontiguousarray(x[b].T).astype(BF),
            "yT": np.ascontiguousarray(y[b].T).astype(BF),
            "maskT": np.ascontiguousarray(mask_f[b].T).astype(BF),
            "wk": np.ascontiguousarray(W_kv[:, k_cols]).astype(BF),
            "wv": np.ascontiguousarray(W_kv[:, v_cols]).astype(BF),
            "wq": np.ascontiguousarray(W_q[:, q_cols]).astype(BF),
            "wo": np.ascontiguousarray(W_o[q_cols, :]).astype(BF),
            "bk": b_kv[k_cols].astype(np.float32).reshape(512, 1),
            "bq": b_q[np.arange(g * 512, (g + 1) * 512)]
                  .astype(np.float32).reshape(512, 1),
        })
    return in_maps


def kernel(x, y, mask, W_kv, b_kv, W_q, b_q, W_o, b_o):
    from concourse import bass_utils

    x = np.asarray(x, np.float32)
    y = np.asarray(y, np.float32)
    mask = np.asarray(mask)
    W_kv = np.asarray(W_kv, np.float32)
    b_kv = np.asarray(b_kv, np.float32)
    W_q = np.asarray(W_q, np.float32)
    b_q = np.asarray(b_q, np.float32)
    W_o = np.asarray(W_o, np.float32)
    b_o = np.asarray(b_o, np.float32)

    nc = _get_program()
    in_maps = _per_core_inputs(x, y, mask, W_kv, b_kv, W_q, b_q, W_o)
    res = bass_utils.run_bass_kernel_spmd(nc, in_maps, core_ids=list(range(8)))

    # b_v folds into a constant row: attn rows sum to 1, so each head adds
    # b_v_h @ W_o_h to every output row; b_o adds on top.
    v_cols_all = (np.arange(H)[:, None] * 2 * HD + HD
                  + np.arange(HD)[None, :]).ravel()
    const_row = b_kv[v_cols_all].astype(np.float32) @ W_o + b_o

    out = np.empty((B, NQ, D), np.float32)
    for b in range(B):
        out[b] = (res.results[2 * b]["out"].astype(np.float32)
                  + res.results[2 * b + 1]["out"].astype(np.float32)
                  + const_row)
    return out


if __name__ == "__main__":
    import reference

    inputs = {k: np.asarray(v) for k, v in reference.setup_inputs().items()}
    got = kernel(**inputs)
    exp = np.asarray(reference.reference(**inputs))
    err = np.abs(got - exp)
    print("absmax rel err:", err.max() / np.abs(exp).max())


# revision 15
# speedup vs baseline: 1.1547x; 1.1547x over previous
"""Multi-head cross-attention Trainium2 Bass kernel, SPMD over 8 NeuronCores.

Sharding: core c handles batch b = c//2 and head group g = c%2 (8 of 16 heads).
Each core computes a partial output projection (its heads' W_o rows); the host
sums the two partials per batch element.

Device pipeline per core (all matmuls bf16 with fp32 PSUM accumulation):
  kT = (Wk^T x^T)          [512 hd, 2048 kseq]   (per-partition bias b_k)
  v  = (x Wv)              [2048 kseq, 8*65]     (65th col per head = ones)
  qT = (Wq^T y^T)          [512 hd, 1024 q]      (per-partition bias b_q)
  per (head-pair, q-tile, k-chunk):
      S^T[k, q|q'] = kT_h^T-chunk @ qT_h for both heads of the pair
        (K=64 row-tiled at partitions 0/64 -> the two matmuls run
         concurrently in the PE array; both write one 2-bank PSUM tile)
      em = exp(0.125 * S^T)  (one ACT op per pair; no row-max: |S|<=~25)
      em *= maskT            (DVE mostly, 1/4 on GPSIMD to balance engines)
      acc_h[65, q] += [v_h | 1]^T @ em_h   (row 64 = softmax denominator)
  normalize: vals_h = acc[0:64] * bcast(1/acc[64])  (PE outer-product bcast,
      reciprocal_approx_fast; deferred into the next head-pair's loop)
  out_partial = vals^T-chunks @ Wo-rows  -> [1024 q, 1024 D] fp32
b_v and b_o fold into a host-side constant row (attn rows sum to 1).

Scheduling: the PE stream (~168us effective) is the pacing resource; ACT
(exp, ~140us) and DVE (~135us) hide underneath IF the pipeline starts early
and stays dense.  Unlike the previous revision (which ran ALL of v/kT0/qT0
before the first score, idling ACT+DVE for ~50us), this version emits only
the minimal prefix (kT0-n0, qT0-n0, v0) and drains every other projection
chain and the first-half output chains through a deadline-driven filler
queue inside the attention loop, so cross-engine overlap begins ~8us in.
DMA issues in need-order with column-sliced weight loads to unblock the
first chains ASAP.  Output copies run on GPSIMD to keep ACT = exp only.
"""

import sys
from collections import deque

import numpy as np
import ml_dtypes

if "/opt/trn_rl_repo" not in sys.path:
    sys.path.insert(0, "/opt/trn_rl_repo")

BF = ml_dtypes.bfloat16

B, NKV, NQ, D, H = 4, 2048, 1024, 1024, 16
HD = D // H          # 64
NHL = 8              # heads per core (local)
P = 128
DC = D // P          # 8 contraction chunks over model dim
KC = NKV // P        # 16 key-seq chunks
QT = NQ // 512       # 2 q tiles of 512 for attention
MT = 4               # hd-dim chunks of kT/qT (512/128)
NSLOT = NHL // 2 * QT * KC   # 128 (hp, t, kc) attention slots

_CACHE = {}


def _build_program():
    import concourse.bass as bass
    import concourse.mybir as mybir
    import concourse.tile as tile
    from concourse import bacc

    f32 = mybir.dt.float32
    bf16 = mybir.dt.bfloat16

    nc = bacc.Bacc(
        "TRN2", target_bir_lowering=False, debug=False, num_devices=8
    )

    xT_d = nc.dram_tensor("xT", [D, NKV], bf16, kind="ExternalInput").ap()
    yT_d = nc.dram_tensor("yT", [D, NQ], bf16, kind="ExternalInput").ap()
    maskT_d = nc.dram_tensor("maskT", [NKV, NQ], bf16, kind="ExternalInput").ap()
    wk_d = nc.dram_tensor("wk", [D, 512], bf16, kind="ExternalInput").ap()
    wv_d = nc.dram_tensor("wv", [D, 512], bf16, kind="ExternalInput").ap()
    wq_d = nc.dram_tensor("wq", [D, 512], bf16, kind="ExternalInput").ap()
    wo_d = nc.dram_tensor("wo", [512, D], bf16, kind="ExternalInput").ap()
    bk_d = nc.dram_tensor("bk", [512, 1], f32, kind="ExternalInput").ap()
    bq_d = nc.dram_tensor("bq", [512, 1], f32, kind="ExternalInput").ap()
    out_d = nc.dram_tensor("out", [NQ, D], bf16, kind="ExternalOutput").ap()

    Exp = mybir.ActivationFunctionType.Exp

    with tile.TileContext(nc) as tc:
        with (
            tc.tile_pool(name="persist", bufs=1) as persist,
            tc.tile_pool(name="work", bufs=3) as work,
            tc.tile_pool(name="empool", bufs=5) as empool,
            tc.tile_pool(name="pmm", bufs=2, space="PSUM") as pmm,
            tc.tile_pool(name="pacc", bufs=2, space="PSUM") as pacc,
            tc.tile_pool(name="psc", bufs=2, space="PSUM") as psc,
        ):
            def row_tile(nchunks, cols, dtype, label):
                return [
                    persist.tile(
                        [P, cols], dtype, tag=f"{label}{i}", name=f"{label}{i}"
                    )
                    for i in range(nchunks)
                ]

            def load(tiles, dram, i):
                dma(tiles[i], dram[i * P:(i + 1) * P, :])

            wk_sb = row_tile(DC, 512, bf16, "wk")
            wv_sb = row_tile(DC, 512, bf16, "wv")
            xT_sb = row_tile(DC, NKV, bf16, "xT")
            wq_sb = row_tile(DC, 512, bf16, "wq")
            yT_sb = row_tile(DC, NQ, bf16, "yT")
            bk_sb = row_tile(MT, 1, f32, "bk")
            bq_sb = row_tile(MT, 1, f32, "bq")
            maskT_sb = row_tile(KC, NQ, bf16, "mT")
            wo_sb = row_tile(MT, D, bf16, "wo")

            # The Sync queue issues one descriptor per ~0.6us, which alone
            # would gate the pipeline ramp (~4.5MB of prefix input needs ~38
            # descriptors).  Round-robin the issue across four otherwise-idle
            # engine queues so the rings fill in parallel.
            dma_engs = [nc.sync, nc.scalar, nc.gpsimd]
            dma_rr = [0]

            def dma(dst, src):
                eng = dma_engs[dma_rr[0] % len(dma_engs)]
                dma_rr[0] += 1
                eng.dma_start(dst, src)

            def load_cols(tiles, dram, d, c0, c1):
                dma(tiles[d][:, c0:c1], dram[d * P:(d + 1) * P, c0:c1])

            # ---- DMA issue order == consumption order ----
            # Prefix (everything the first ~12 slots touch) issues
            # round-robin across the four queues while the compute engines
            # are still empty; the rest goes to the Sync queue alone, in
            # urgency order (masks and xT quarters first), so no descriptor
            # issue ever blocks a busy compute sequencer.
            for d in range(DC):
                load_cols(wk_sb, wk_d, d, 0, P)       # wk m0 cols only
                load_cols(xT_sb, xT_d, d, 0, 512)     # xT q-quarter 0
            for d in range(DC):
                load_cols(wq_sb, wq_d, d, 0, P)       # wq m0 cols only
                load_cols(yT_sb, yT_d, d, 0, 512)     # yT first q-half
            for d in range(DC):
                load(wv_sb, wv_d, d)                  # wv full (v chains)
            for m in range(MT):
                load(bk_sb, bk_d, m)
                load(bq_sb, bq_d, m)
            for i in range(0, 2):
                load(maskT_sb, maskT_d, i)
            for d in range(DC):
                load_cols(xT_sb, xT_d, d, 512, 1024)  # xT quarter 1
            dma_engs[:] = [nc.sync]                    # post-prefix: Sync only
            for i in range(2, 6):
                load(maskT_sb, maskT_d, i)
            for d in range(DC):
                load_cols(xT_sb, xT_d, d, 1024, 1536)  # xT quarter 2
            for i in range(6, 10):
                load(maskT_sb, maskT_d, i)
            for d in range(DC):
                load_cols(xT_sb, xT_d, d, 1536, 2048)  # xT quarter 3
            for i in range(10, 16):
                load(maskT_sb, maskT_d, i)
            for d in range(DC):
                load_cols(wk_sb, wk_d, d, P, 512)     # wk m1-3
            for d in range(DC):
                load_cols(wq_sb, wq_d, d, P, 512)     # wq m1-3
            for d in range(DC):
                load_cols(yT_sb, yT_d, d, 512, 1024)  # yT second q-half
            for m in range(MT):
                load(wo_sb, wo_d, m)

            ones_sb = persist.tile([1, HD], bf16, tag="ones", name="ones")
            nc.gpsimd.memset(ones_sb, 1.0)

            kT_sb = [
                persist.tile([P, NKV], bf16, tag=f"kT{m}", name=f"kT{m}")
                for m in range(MT)
            ]
            qT_sb = [
                persist.tile([P, NQ], bf16, tag=f"qT{m}", name=f"qT{m}")
                for m in range(MT)
            ]
            v_sb = [
                persist.tile([P, NHL * 65], bf16, tag=f"v{i}", name=f"v{i}")
                for i in range(KC)
            ]
            for i in range(KC):
                nc.gpsimd.memset(
                    v_sb[i].rearrange("p (h c) -> p h c", c=65)[:, :, 64:65], 1.0
                )
            vals_sb = [
                persist.tile([P, NQ], bf16, tag=f"vals{c}", name=f"vals{c}")
                for c in range(MT)
            ]

            # ---- chain builders (lists of single-matmul closures) ----
            def proj_chain(m, n, which):
                """kT/qT chunk-m column-chain n: 8 matmuls + bias-add."""
                w_sb, dst, bias = (
                    (wk_sb, kT_sb, bk_sb) if which == "k"
                    else (wq_sb, qT_sb, bq_sb)
                )
                hold = {}
                ops = []
                for d in range(DC):
                    def op(d=d):
                        if d == 0:
                            hold[0] = pmm.tile(
                                [P, 512], f32, tag="mm",
                                name=f"pj{which}{m}_{n}"
                            )
                        nc.tensor.matmul(
                            hold[0],
                            lhsT=w_sb[d][:, m * P:(m + 1) * P],
                            rhs=(xT_sb if which == "k" else yT_sb)[d][
                                :, n * 512:(n + 1) * 512],
                            start=(d == 0),
                            stop=(d == DC - 1),
                        )
                        if d == DC - 1:
                            nc.vector.tensor_scalar_add(
                                dst[m][:, n * 512:(n + 1) * 512],
                                hold[0], bias[m]
                            )
                    ops.append(op)
                return ops

            def v_chain(i):
                """v row-chunk i: 8 matmuls + PSUM->SBUF copy."""
                hold = {}
                ops = []
                for d in range(DC):
                    def op(d=d, i=i):
                        if d == 0:
                            hold[0] = pmm.tile(
                                [P, 512], f32, tag="mm", name=f"ps_v{i}"
                            )
                        nc.tensor.matmul(
                            hold[0],
                            lhsT=xT_sb[d][:, i * P:(i + 1) * P],
                            rhs=wv_sb[d],
                            start=(d == 0),
                            stop=(d == DC - 1),
                        )
                        if d == DC - 1:
                            v3 = v_sb[i].rearrange("p (h c) -> p h c", c=65)
                            nc.vector.tensor_copy(
                                v3[:, :, 0:64],
                                hold[0].rearrange("p (h c) -> p h c", c=64),
                            )
                    ops.append(op)
                return ops

            def wo_chain(t2, n):
                """output-projection chain: 4 matmuls + copy (GPSIMD) + DMA."""
                hold = {}
                ops = []
                for c in range(MT):
                    def op(c=c, t2=t2, n=n):
                        if c == 0:
                            hold[0] = pmm.tile(
                                [P, 512], f32, tag="mm", name=f"ps_o{t2}_{n}"
                            )
                        nc.tensor.matmul(
                            hold[0],
                            lhsT=vals_sb[c][:, t2 * P:(t2 + 1) * P],
                            rhs=wo_sb[c][:, n * 512:(n + 1) * 512],
                            start=(c == 0),
                            stop=(c == MT - 1),
                        )
                        if c == MT - 1:
                            ot = work.tile(
                                [P, 512], bf16, tag="ot",
                                name=f"ot{t2}_{n}", bufs=3
                            )
                            # GPSIMD cannot read PSUM; DVE has the headroom
                            nc.vector.tensor_copy(ot, hold[0])
                            nc.sync.dma_start(
                                out_d[t2 * P:(t2 + 1) * P,
                                      n * 512:(n + 1) * 512], ot
                            )
                    ops.append(op)
                return ops

            # ---- deadline-sorted filler queue ----
            # Each entry: (deadline_slot, op).  Popped (and emitted) when the
            # attention loop reaches that slot; order within equal deadlines
            # follows append order, which is dependency-safe.
            pend = []

            def add_chain(deadline, ops):
                for op in ops:
                    pend.append((deadline, op))

            # v[i] first consumed at slot i (block 0 covers kc 0..15); its
            # attn matmul trails the slot's scores by ~1.5 slots of
            # cross-engine latency, so deadline i still arrives in time
            for i in range(1, KC):
                add_chain(i, v_chain(i))
            # kT chunk 0 column-chains n>=1: consumed at slot 4n
            for n in range(1, 4):
                add_chain(4 * n - 2, proj_chain(0, n, "k"))
            # qT chunk 0 n=1: consumed from slot 16
            add_chain(13, proj_chain(0, 1, "q"))
            # chunks m>=1: block (m, t) starts at slot 32m + 16t
            for m in range(1, MT):
                for n in range(4):
                    add_chain(32 * m + 4 * n - 3, proj_chain(m, n, "k"))
                for t in range(QT):
                    add_chain(32 * m + 16 * t - 3, proj_chain(m, t, "q"))
            # first-half output chains: vals q<512 final once block (3,0)'s
            # norms have popped (slots 112 and 114 of block (3,1)); spread
            # the 8 chains through the rest of block (3,1)
            for j, (t2, n) in enumerate(
                [(t2, n) for t2 in range(4) for n in range(2)]
            ):
                add_chain(115 + (3 * j) // 2, wo_chain(t2, n))
            pend.sort(key=lambda e: e[0])
            pend = deque(pend)

            def drain(s):
                while pend and pend[0][0] <= s:
                    pend.popleft()[1]()

            # ---- minimal prefix: first scores/attn inputs ----
            for op in proj_chain(0, 0, "k"):
                op()
            for op in proj_chain(0, 0, "q"):
                op()
            for op in v_chain(0):
                op()

            # ---- attention (fillers + deferred normalize interleaved) ----
            norm_pending = deque()

            def make_norm(hp, t, a, h, ut, s_f):
                po = a * HD
                qs = slice(t * 512, (t + 1) * 512)

                def norm_op():
                    r_f = work.tile([1, 512], f32, tag="r", name=f"r{h}_{t}")
                    nc.vector.reciprocal_approx_fast(r_f, s_f)
                    r_b = work.tile([1, 512], bf16, tag="rb", name=f"rb{h}_{t}")
                    nc.vector.tensor_copy(r_b, r_f)
                    bps = pmm.tile([HD, 512], f32, tag="mm", name=f"bps{h}_{t}")
                    nc.tensor.matmul(
                        bps, lhsT=ones_sb, rhs=r_b, start=True, stop=True
                    )
                    nc.vector.tensor_mul(vals_sb[hp][po:po + HD, qs], ut, bps)
                return norm_op

            ALAG = 2  # slots the attn matmuls trail the scores/exp/mask ops

            for hp in range(NHL // 2):
                h0, h1 = 2 * hp, 2 * hp + 1
                for t in range(QT):
                    qs = slice(t * 512, (t + 1) * 512)
                    accs = [
                        pacc.tile([65, 512], f32, tag="acc", name=f"acc{h}_{t}")
                        for h in (h0, h1)
                    ]
                    attn_q = deque()
                    for kc in range(KC):
                        s = (hp * QT + t) * KC + kc
                        sp2 = psc.tile(
                            [P, 1024], f32, tag="sc", name=f"sp{hp}_{t}_{kc}"
                        )
                        for a in range(2):
                            po = a * HD
                            nc.tensor.matmul(
                                sp2[:, a * 512:(a + 1) * 512],
                                lhsT=kT_sb[hp][po:po + HD, kc * P:(kc + 1) * P],
                                rhs=qT_sb[hp][po:po + HD, qs],
                                start=True,
                                stop=True,
                            )
                        em2 = empool.tile(
                            [P, 1024], bf16, tag="em", name=f"em{hp}_{t}_{kc}"
                        )
                        nc.scalar.activation(em2, sp2, Exp, scale=0.125)
                        # one masked multiply for both heads: the mask chunk is
                        # read once and broadcast (step-0 dim) over the pair;
                        # odd chunks (except the last two, whose attn flushes
                        # at the block boundary) run on GPSIMD to split the
                        # heaviest element-wise load across two engines
                        mb = (maskT_sb[kc][:, qs]
                              .rearrange("p (o q) -> p o q", o=1)
                              .broadcast_to([P, 2, 512]))
                        em3 = em2.rearrange("p (o q) -> p o q", o=2)
                        if kc % 2 == 1 and kc < KC - 2:
                            nc.gpsimd.tensor_mul(em3, em3, mb)
                        else:
                            nc.vector.tensor_mul(em3, em3, mb)

                        def attn_op(kc=kc, em2=em2):
                            for a, h in enumerate((h0, h1)):
                                nc.tensor.matmul(
                                    accs[a],
                                    lhsT=v_sb[kc][:, h * 65:(h + 1) * 65],
                                    rhs=em2[:, a * 512:(a + 1) * 512],
                                    start=(kc == 0),
                                    stop=(kc == KC - 1),
                                )
                        attn_q.append(attn_op)

                        # fillers and the lagged attn matmuls go AFTER the
                        # mask op: the in-order PE works through fillers
                        # while DVE/GPSIMD finish masks, and an attn matmul
                        # only reaches the PE ~2 slots after its em tile was
                        # produced, so even a slow GPSIMD mask never stalls
                        # the engine
                        drain(s)
                        if kc in (0, 2) and norm_pending:
                            norm_pending.popleft()()
                        if len(attn_q) > ALAG:
                            attn_q.popleft()()
                    while attn_q:
                        attn_q.popleft()()
                    for a, h in enumerate((h0, h1)):
                        acc = accs[a]
                        # free the PSUM accumulator quickly (copies only);
                        # the rest of the normalization is deferred into the
                        # next block's loop.
                        ut = work.tile(
                            [HD, 512], f32, tag="ut", name=f"ut{h}_{t}", bufs=5
                        )
                        nc.vector.tensor_copy(ut, acc[0:HD, :])
                        s_f = work.tile(
                            [1, 512], f32, tag="s", name=f"s{h}_{t}", bufs=5
                        )
                        nc.scalar.copy(s_f, acc[64:65, :])
                        norm_pending.append(make_norm(hp, t, a, h, ut, s_f))

            # ---- tail: last block's norms, then second-half out chains ----
            while norm_pending:
                norm_pending.popleft()()
            drain(NSLOT)               # any stragglers (should be none)
            for t2 in range(4, NQ // P):
                for n in range(2):
                    for op in wo_chain(t2, n):
                        op()

    nc.compile()
    return nc


def _get_program():
    if "nc" not in _CACHE:
        _CACHE["nc"] = _build_program()
    return _CACHE["nc"]


def _per_core_inputs(x, y, mask, W_kv, b_kv, W_q, b_q, W_o):
    """Build the 8 per-core input maps."""
    in_maps = []
    mask_f = mask.astype(np.float32)
    for c in range(8):
        b, g = c // 2, c % 2
        gh = np.arange(g * 8, g * 8 + 8)
        k_cols = (gh[:, None] * 2 * HD + np.arange(HD)[None, :]).ravel()
        v_cols = k_cols + HD
        q_cols = slice(g * 512, (g + 1) * 512)
        in_maps.append({
            "xT": np.ascontiguousarray(x[b].T).astype(BF),
            "yT": np.ascontiguousarray(y[b].T).astype(BF),
            "maskT": np.ascontiguousarray(mask_f[b].T).astype(BF),
            "wk": np.ascontiguousarray(W_kv[:, k_cols]).astype(BF),
            "wv": np.ascontiguousarray(W_kv[:, v_cols]).astype(BF),
            "wq": np.ascontiguousarray(W_q[:, q_cols]).astype(BF),
            "wo": np.ascontiguousarray(W_o[q_cols, :]).astype(BF),
            "bk": b_kv[k_cols].astype(np.float32).reshape(512, 1),
            "bq": b_q[np.arange(g * 512, (g + 1) * 512)]
                  .astype(np.float32).reshape(512, 1),
        })
    return in_maps


def kernel(x, y, mask, W_kv, b_kv, W_q, b_q, W_o, b_o):
    from concourse import bass_utils

    x = np.asarray(x, np.float32)
    y = np.asarray(y, np.float32)
    mask = np.asarray(mask)
    W_kv = np.asarray(W_kv, np.float32)
    b_kv = np.asarray(b_kv, np.float32)
    W_q = np.asarray(W_q, np.float32)
    b_q = np.asarray(b_q, np.float32)
    W_o = np.asarray(W_o, np.float32)
    b_o = np.asarray(b_o, np.float32)

    nc = _get_program()
    in_maps = _per_core_inputs(x, y, mask, W_kv, b_kv, W_q, b_q, W_o)
    res = bass_utils.run_bass_kernel_spmd(nc, in_maps, core_ids=list(range(8)))

    # b_v folds into a constant row: attn rows sum to 1, so each head adds
    # b_v_h @ W_o_h to every output row; b_o adds on top.
    v_cols_all = (np.arange(H)[:, None] * 2 * HD + HD
                  + np.arange(HD)[None, :]).ravel()
    const_row = b_kv[v_cols_all].astype(np.float32) @ W_o + b_o

    out = np.empty((B, NQ, D), np.float32)
    for b in range(B):
        out[b] = (res.results[2 * b]["out"].astype(np.float32)
                  + res.results[2 * b + 1]["out"].astype(np.float32)
                  + const_row)
    return out


if __name__ == "__main__":
    import reference

    inputs = {k: np.asarray(v) for k, v in reference.setup_inputs().items()}
    got = kernel(**inputs)
    exp = np.asarray(reference.reference(**inputs))
    err = np.abs(got - exp)
    print("absmax rel err:", err.max() / np.abs(exp).max())
